# revision 1
# baseline (speedup 1.0000x reference)
"""Trainium2 Bass kernel for a GCN layer (gnn_message_passing).

Reference computation (per batch b):
    node_msg = h @ Wn_w.T + Wn_b                  # (N, OD)
    edge_msg = edge_feat @ We_w.T + We_b          # (N, N, OD)
    self_msg = h @ Ws_w.T + Ws_b                  # (N, OD)
    deg      = adj.sum(-1)                        # (N,)
    agg      = node_msg * deg + einsum('ij,ijo->io', adj, edge_msg)
    out      = relu(agg / clip(deg, 1) + self_msg)

Key algebraic rewrite: the (N,N,OD) edge_msg is never materialized.
    einsum('ij,ijo->io', adj, edge_feat @ We.T + We_b)
      = (einsum('ij,ije->ie', adj, edge_feat)) @ We.T + deg * We_b
so the dominant work is the adj-masked reduction of edge_feat over the
source-node axis j, producing (N, ED), followed by a tiny 16->64 matmul.

Sharding: data-parallel over batch B=8 across the 8 NeuronCores (one
batch element per core); weights replicated.

On-chip strategy per core:
  - edge_feat[b] is streamed in i-blocks of 128 destination nodes as
    (128p, 512j, 16e) tiles: 32 KB contiguous per partition (ideal DMA).
  - masked reduce uses the fused DVE op scalar_tensor_tensor with
    accum_out (TENSOR_SCALAR_PTR; note tensor_tensor_reduce faults the
    device on this platform):
        accum_out[i, e] = sum_j ef[i, j, e] * adj_f[i, j]
    two FD=256 ops per e-channel per block -- a single DVE pass over
    the data instead of separate multiply + reduce passes.
  - the (128, 16) masked sums are PE-transposed and projected with We^T
    on the TensorEngine; node/self messages are matmuls with the bias
    folded in via an appended ones-row on h^T.
"""

import os
import sys
from contextlib import ExitStack

import numpy as np


def _ensure_concourse():
    try:
        import concourse  # noqa: F401
        return
    except ImportError:
        pass
    for p in ("/opt/trn_rl_repo", "/root/.axon_site/_ro/trn_rl_repo"):
        if os.path.isdir(p) and p not in sys.path:
            sys.path.insert(0, p)
            try:
                import concourse  # noqa: F401
                return
            except ImportError:
                continue
    raise ImportError("cannot locate the concourse (bass) package")


_ensure_concourse()

import concourse.bacc as bacc  # noqa: E402
import concourse.bass as bass  # noqa: E402
import concourse.tile as tile  # noqa: E402
from concourse import mybir  # noqa: E402
from concourse.bass_utils import run_bass_kernel_spmd  # noqa: E402
from concourse.masks import make_identity  # noqa: E402

B, N, ND, ED, OD = 8, 512, 64, 16, 64
NCORES = 8
PB = 128           # destination-node block (SBUF partitions)
NBLK = N // PB     # 4

F32 = mybir.dt.float32
I32 = mybir.dt.int32


def _row_ap(handle, n):
    """View a 1-D DRAM tensor (n,) as a (1, n) AP."""
    ap = handle.ap()
    return bass.AP(tensor=ap.tensor, offset=ap.offset, ap=[[0, 1], [1, n]])


def build_bass(repeat=1, mode="full", unroll=1):
    """Build the single-core Bass program (SPMD across 8 cores).

    repeat>1 wraps the whole body in an on-device For_i loop -- used only
    for timing (amortizes host dispatch overhead away).
    mode: "full" | "dma_only" (1 fused-reduce per block) | "dve_only"
    (ef loaded once, reused by all blocks) -- benchmarking diagnostics.
    """
    nc = bacc.Bacc(
        "TRN2",
        target_bir_lowering=False,
        debug=False,
        num_devices=NCORES,
    )

    h_d = nc.dram_tensor("h", [N, ND], F32, kind="ExternalInput")
    adj_d = nc.dram_tensor("adj", [N, N], I32, kind="ExternalInput")
    ef_d = nc.dram_tensor("edge_feat", [N, N, ED], F32, kind="ExternalInput")
    wn_d = nc.dram_tensor("Wn_w", [OD, ND], F32, kind="ExternalInput")
    wnb_d = nc.dram_tensor("Wn_b", [OD], F32, kind="ExternalInput")
    we_d = nc.dram_tensor("We_w", [OD, ED], F32, kind="ExternalInput")
    web_d = nc.dram_tensor("We_b", [OD], F32, kind="ExternalInput")
    ws_d = nc.dram_tensor("Ws_w", [OD, ND], F32, kind="ExternalInput")
    wsb_d = nc.dram_tensor("Ws_b", [OD], F32, kind="ExternalInput")
    out_d = nc.dram_tensor("out", [N, OD], F32, kind="ExternalOutput")

    h_ap = h_d.ap()
    adj_ap = adj_d.ap()
    ef_ap = ef_d.ap()
    out_ap = out_d.ap()

    with tile.TileContext(nc) as tc, ExitStack() as ctx:
        consts = ctx.enter_context(tc.tile_pool(name="consts", bufs=1))
        efp = ctx.enter_context(
            tc.tile_pool(name="efp", bufs=2 if mode == "hybe" else 4)
        )
        prodp = (
            ctx.enter_context(tc.tile_pool(name="prodp", bufs=2))
            if mode == "hybe"
            else None
        )
        adjp = ctx.enter_context(tc.tile_pool(name="adjp", bufs=1))
        work = ctx.enter_context(tc.tile_pool(name="work", bufs=2))
        outp = ctx.enter_context(tc.tile_pool(name="outp", bufs=2))
        pset = ctx.enter_context(tc.tile_pool(name="pset", bufs=1, space="PSUM"))
        pmm = ctx.enter_context(tc.tile_pool(name="pmm", bufs=1, space="PSUM"))
        pms = ctx.enter_context(tc.tile_pool(name="pms", bufs=2, space="PSUM"))

        def emit_body():
            ident = consts.tile([128, 128], F32)
            make_identity(nc, ident)

            # --- weights: transpose on PE; biases folded as extra matmul row ---
            # rhs_n = [Wn_w^T ; Wn_b + We_b]  (65, 64)
            # rhs_s = [Ws_w^T ; Ws_b]         (65, 64)
            # weT   = We_w^T                  (16, 64)
            wn_sb = consts.tile([OD, ND], F32, tag="wload")
            nc.scalar.dma_start(out=wn_sb, in_=wn_d.ap())
            ws_sb = consts.tile([OD, ND], F32, tag="wload2")
            nc.scalar.dma_start(out=ws_sb, in_=ws_d.ap())
            we_sb = consts.tile([OD, ED], F32, tag="wload3")
            nc.scalar.dma_start(out=we_sb, in_=we_d.ap())

            rhs_n = consts.tile([ND + 1, OD], F32)
            rhs_s = consts.tile([ND + 1, OD], F32)
            weT = consts.tile([ED, OD], F32)

            pw = pset.tile([ND, OD], F32, tag="pw")
            nc.tensor.transpose(pw, wn_sb, ident[:ND, :OD])
            nc.scalar.copy(out=rhs_n[0:ND, :], in_=pw)
            pw2 = pset.tile([ND, OD], F32, tag="pw")
            nc.tensor.transpose(pw2, ws_sb, ident[:ND, :OD])
            nc.scalar.copy(out=rhs_s[0:ND, :], in_=pw2)
            pw3 = pset.tile([ED, OD], F32, tag="pw")
            nc.tensor.transpose(pw3, we_sb, ident[:ND, :OD])
            nc.scalar.copy(out=weT, in_=pw3)

            bias_n = consts.tile([1, OD], F32)
            nc.scalar.dma_start(out=bias_n, in_=_row_ap(wnb_d, OD))
            bias_e = consts.tile([1, OD], F32)
            nc.scalar.dma_start(out=bias_e, in_=_row_ap(web_d, OD))
            nc.vector.tensor_add(rhs_n[ND : ND + 1, :], bias_n, bias_e)
            nc.scalar.dma_start(out=rhs_s[ND : ND + 1, :], in_=_row_ap(wsb_d, OD))

            # --- h^T with an appended ones-row: (65, 512) ---
            hT = consts.tile([ND + 1, N], F32)
            nc.vector.memset(hT[ND : ND + 1, :], 1.0)
            for ib in range(NBLK):
                h_sb = work.tile([PB, ND], F32, tag="hload")
                nc.scalar.dma_start(out=h_sb, in_=h_ap[ib * PB : (ib + 1) * PB, :])
                ph = pset.tile([ND, PB], F32, tag="ph")
                nc.tensor.transpose(ph, h_sb, ident)
                nc.scalar.copy(out=hT[0:ND, ib * PB : (ib + 1) * PB], in_=ph)

            # --- adj / degree prep for all blocks (off the ef critical path;
            #     scalar-engine HWDGE ring, distinct from the ef ring) ---
            adj_fs, rs, degrs = [], [], []
            for ib in range(NBLK):
                i0 = ib * PB
                adj_i = adjp.tile([PB, N], I32, tag=f"adji{ib}")
                nc.scalar.dma_start(out=adj_i, in_=adj_ap[i0 : i0 + PB, :])
                adj_f = adjp.tile([PB, N], F32, tag=f"adjf{ib}")
                nc.vector.tensor_copy(out=adj_f, in_=adj_i)

                deg = work.tile([PB, 1], F32, tag=f"deg{ib}")
                nc.vector.reduce_sum(deg, adj_f, axis=mybir.AxisListType.X)
                degc = work.tile([PB, 1], F32, tag=f"degc{ib}")
                nc.vector.tensor_scalar_max(degc, deg, 1.0)
                r = work.tile([PB, 1], F32, tag=f"r{ib}")
                nc.vector.reciprocal(r, degc)
                degr = work.tile([PB, 1], F32, tag=f"degr{ib}")
                nc.vector.tensor_mul(degr, deg, r)
                adj_fs.append(adj_f)
                rs.append(r)
                degrs.append(degr)

            def _stt_path(ef_t, adj_f, r, mode):
                # masked sum over source nodes j, fused multiply+reduce on
                # DVE (scalar_tensor_tensor accum_out), split in j-halves.
                ms_a = work.tile([PB, ED], F32, tag="msa")
                ms_b = work.tile([PB, ED], F32, tag="msb")
                scratch = work.tile([PB, N // 2], F32, tag="scratch")
                HJ = N // 2
                n_e = 1 if mode == "dma_only" else ED
                for e in range(n_e):
                    nc.vector.scalar_tensor_tensor(
                        out=scratch,
                        in0=ef_t[:, 0:HJ, e],
                        scalar=1.0,
                        in1=adj_f[:, 0:HJ],
                        op0=mybir.AluOpType.bypass,
                        op1=mybir.AluOpType.mult,
                        accum_out=ms_a[:, e : e + 1],
                    )
                for e in range(n_e):
                    nc.vector.scalar_tensor_tensor(
                        out=scratch,
                        in0=ef_t[:, HJ:N, e],
                        scalar=1.0,
                        in1=adj_f[:, HJ:N],
                        op0=mybir.AluOpType.bypass,
                        op1=mybir.AluOpType.mult,
                        accum_out=ms_b[:, e : e + 1],
                    )
                msum = work.tile([PB, ED], F32, tag="msum")
                nc.vector.tensor_add(msum, ms_a, ms_b)
                ms = work.tile([PB, ED], F32, tag="ms")
                nc.vector.tensor_scalar_mul(ms, msum, r)
                return ms

            # --- main loop over destination-node blocks ---
            ef_shared = None
            for ib in range(NBLK):
                i0 = ib * PB
                adj_f, r, degr = adj_fs[ib], rs[ib], degrs[ib]

                HJ2 = N // 2
                if mode == "dve_only":
                    if ef_shared is None:
                        ef_shared = efp.tile([PB, N, ED], F32, tag="ef")
                        nc.sync.dma_start(out=ef_shared, in_=ef_ap[0:PB, :, :])
                    ef_t = ef_shared
                else:
                    # j-halves as separate DMAs: the first-half fused-reduce
                    # ops depend only on the first 2 MB, so DVE starts while
                    # the second half is still in flight.
                    ef_t = efp.tile([PB, N, ED], F32, tag="ef")
                    nc.sync.dma_start(
                        out=ef_t[:, 0:HJ2, :], in_=ef_ap[i0 : i0 + PB, 0:HJ2, :]
                    )
                    nc.sync.dma_start(
                        out=ef_t[:, HJ2:N, :], in_=ef_ap[i0 : i0 + PB, HJ2:N, :]
                    )

                if mode == "hybe":
                    # hybrid: DVE does ONE contiguous broadcast-mask multiply
                    # per j-half, writing the product e-major; ACT reduces
                    # each e-row (contiguous) via activation accum_out.
                    prod_em = prodp.tile([PB, ED, N], F32, tag="prod")
                    for jh in range(2):
                        h0 = jh * HJ2
                        in0 = ef_t[:, h0 : h0 + HJ2, :]
                        base = adj_f[:, h0 : h0 + HJ2]
                        in1 = bass.AP(
                            tensor=base.tensor,
                            offset=base.offset,
                            ap=list(base.ap) + [[0, ED]],
                        )
                        prod_out = bass.AP(
                            tensor=prod_em.tensor,
                            offset=prod_em.offset + h0,
                            ap=[prod_em.ap[0], [1, HJ2], [N, ED]],
                        )
                        nc.vector.tensor_tensor(
                            out=prod_out, in0=in0, in1=in1, op=mybir.AluOpType.mult
                        )
                    ms_h = work.tile([PB, ED], F32, tag="msh")
                    act_scr = work.tile([PB, N], F32, tag="actscr")
                    for e in range(ED):
                        nc.scalar.activation(
                            out=act_scr,
                            in_=prod_em[:, e, :],
                            func=mybir.ActivationFunctionType.Copy,
                            accum_out=ms_h[:, e : e + 1],
                        )
                    ms = work.tile([PB, ED], F32, tag="ms")
                    nc.vector.tensor_scalar_mul(ms, ms_h, r)
                else:
                    ms = _stt_path(ef_t, adj_f, r, mode)

                # (128, 16) -> (16, 128) for the We projection
                pm = pms.tile([ED, PB], F32, tag="pm")
                nc.tensor.transpose(pm, ms, ident)
                msT = work.tile([ED, PB], F32, tag="msT")
                nc.scalar.copy(out=msT, in_=pm)

                # psum_es = (r*ms)^T We^T + h Ws^T + Ws_b   (PSUM accumulate)
                pes = pmm.tile([PB, OD], F32, tag="pes")
                nc.tensor.matmul(pes, lhsT=msT, rhs=weT, start=True, stop=False)
                nc.tensor.matmul(
                    pes, lhsT=hT[:, i0 : i0 + PB], rhs=rhs_s, start=False, stop=True
                )
                pn = pmm.tile([PB, OD], F32, tag="pn")
                nc.tensor.matmul(
                    pn, lhsT=hT[:, i0 : i0 + PB], rhs=rhs_n, start=True, stop=True
                )

                # out = relu(degr * node + pes)
                acc = outp.tile([PB, OD], F32, tag="acc")
                nc.vector.tensor_scalar_mul(acc, pn, degr)
                ob = outp.tile([PB, OD], F32, tag="ob")
                nc.vector.scalar_tensor_tensor(
                    out=ob,
                    in0=pes,
                    scalar=1.0,
                    in1=acc,
                    op0=mybir.AluOpType.bypass,
                    op1=mybir.AluOpType.add,
                )
                nc.scalar.activation(
                    out=ob, in_=ob, func=mybir.ActivationFunctionType.Relu
                )
                nc.scalar.dma_start(out=out_ap[i0 : i0 + PB, :], in_=ob)


        if repeat == 1:
            for _ in range(unroll):
                emit_body()
        else:
            with tc.For_i(0, repeat, 1):
                for _ in range(unroll):
                    emit_body()

    nc.compile()
    return nc


_NC_CACHE = None


def _get_nc():
    global _NC_CACHE
    if _NC_CACHE is None:
        _NC_CACHE = build_bass()
    return _NC_CACHE


def make_in_maps(inputs):
    w = {
        k: np.ascontiguousarray(np.asarray(inputs[k], dtype=np.float32))
        for k in ("Wn_w", "Wn_b", "We_w", "We_b", "Ws_w", "Ws_b")
    }
    h = np.asarray(inputs["h"], dtype=np.float32)
    adj = np.asarray(inputs["adj"], dtype=np.int32)
    ef = np.asarray(inputs["edge_feat"], dtype=np.float32)
    in_maps = []
    for c in range(NCORES):
        m = dict(w)
        m["h"] = np.ascontiguousarray(h[c])
        m["adj"] = np.ascontiguousarray(adj[c])
        m["edge_feat"] = np.ascontiguousarray(ef[c])
        in_maps.append(m)
    return in_maps


def run(inputs, trace=False):
    """Run on hardware; returns (full_output, BassKernelResults)."""
    nc = _get_nc()
    res = run_bass_kernel_spmd(nc, make_in_maps(inputs), list(range(NCORES)), trace=trace)
    out = np.stack(
        [np.asarray(res.results[c]["out"]) for c in range(NCORES)], axis=0
    ).astype(np.float32)
    return out, res


def kernel(**inputs):
    out, _ = run(inputs)
    return out



# revision 14
# speedup vs baseline: 9.2492x; 9.2492x over previous
"""Trainium2 Bass kernel for a GCN layer (gnn_message_passing).

Reference computation (per batch b):
    node_msg = h @ Wn_w.T + Wn_b                  # (N, OD)
    edge_msg = edge_feat @ We_w.T + We_b          # (N, N, OD)
    self_msg = h @ Ws_w.T + Ws_b                  # (N, OD)
    deg      = adj.sum(-1)                        # (N,)
    agg      = node_msg * deg + einsum('ij,ijo->io', adj, edge_msg)
    out      = relu(agg / clip(deg, 1) + self_msg)

Key algebraic rewrite: the (N,N,OD) edge_msg is never materialized.
    einsum('ij,ijo->io', adj, edge_feat @ We.T + We_b)
      = (einsum('ij,ije->ie', adj, edge_feat)) @ We.T + deg * We_b
so the dominant work is the adj-masked reduction of edge_feat over the
source-node axis j, producing (N, ED), followed by a tiny 16->64 matmul.

Sharding: data-parallel over batch B=8 across the 8 NeuronCores (one
batch element per core); weights replicated.

On-chip strategy per core:
  - edge_feat[b] is streamed in i-blocks of 128 destination nodes as
    (128p, 512j, 16e) tiles in 2 MB j-halves: 32 KB contiguous per
    partition (ideal DMA), and small transfers are never stuck behind
    more than ~5.8 us of bulk data on the shared DMA engines.
  - masked reduce uses the fused DVE op scalar_tensor_tensor with
    accum_out (TENSOR_SCALAR_PTR; tensor_tensor_reduce faults the
    device on this platform):
        accum_out[i, e] = sum_j ef[i, j, e] * adj_f[i, j]
    one FD=512 op per e-channel per block (16 ops instead of 32): the
    ~290-cycle per-op SBUF access overhead amortizes over the full
    row, cutting DVE busy from ~48 us to ~41 us, below the ~51 us DMA
    floor (17.3 MB/core at ~358 GB/s HBM-per-core).
  - degree reduction (adj row-sums) and the pn*degr scaling run on the
    ACT engine (activation accum_out / per-partition scale), off the
    DVE critical path.
  - the (128, 16) masked sums are PE-transposed and projected with We^T
    on the TensorEngine; node/self messages are matmuls with the bias
    folded in via an appended ones-row on h^T.
"""

import os
import sys
from contextlib import ExitStack

import numpy as np


def _ensure_concourse():
    try:
        import concourse  # noqa: F401
        return
    except ImportError:
        pass
    for p in ("/opt/trn_rl_repo", "/root/.axon_site/_ro/trn_rl_repo"):
        if os.path.isdir(p) and p not in sys.path:
            sys.path.insert(0, p)
            try:
                import concourse  # noqa: F401
                return
            except ImportError:
                continue
    raise ImportError("cannot locate the concourse (bass) package")


_ensure_concourse()

import concourse.bacc as bacc  # noqa: E402
import concourse.bass as bass  # noqa: E402
import concourse.tile as tile  # noqa: E402
from concourse import mybir  # noqa: E402
from concourse.bass_utils import run_bass_kernel_spmd  # noqa: E402
from concourse.masks import make_identity  # noqa: E402

B, N, ND, ED, OD = 8, 512, 64, 16, 64
NCORES = 8
PB = 128           # destination-node block (SBUF partitions)
NBLK = N // PB     # 4

F32 = mybir.dt.float32
I32 = mybir.dt.int32


def _row_ap(handle, n):
    """View a 1-D DRAM tensor (n,) as a (1, n) AP."""
    ap = handle.ap()
    return bass.AP(tensor=ap.tensor, offset=ap.offset, ap=[[0, 1], [1, n]])


def build_bass(repeat=1, mode="full", unroll=1):
    """Build the single-core Bass program (SPMD across 8 cores).

    repeat>1 wraps the whole body in an on-device For_i loop -- used only
    for timing (amortizes host dispatch overhead away).  mode accepted for
    bench compatibility ("full" only).
    """
    nc = bacc.Bacc(
        "TRN2",
        target_bir_lowering=False,
        debug=False,
        num_devices=NCORES,
    )

    h_d = nc.dram_tensor("h", [N, ND], F32, kind="ExternalInput")
    adj_d = nc.dram_tensor("adj", [N, N], I32, kind="ExternalInput")
    ef_d = nc.dram_tensor("edge_feat", [N, N, ED], F32, kind="ExternalInput")
    wn_d = nc.dram_tensor("Wn_w", [OD, ND], F32, kind="ExternalInput")
    wnb_d = nc.dram_tensor("Wn_b", [OD], F32, kind="ExternalInput")
    we_d = nc.dram_tensor("We_w", [OD, ED], F32, kind="ExternalInput")
    web_d = nc.dram_tensor("We_b", [OD], F32, kind="ExternalInput")
    ws_d = nc.dram_tensor("Ws_w", [OD, ND], F32, kind="ExternalInput")
    wsb_d = nc.dram_tensor("Ws_b", [OD], F32, kind="ExternalInput")
    out_d = nc.dram_tensor("out", [N, OD], F32, kind="ExternalOutput")

    h_ap = h_d.ap()
    adj_ap = adj_d.ap()
    ef_ap = ef_d.ap()
    out_ap = out_d.ap()

    with tile.TileContext(nc) as tc, ExitStack() as ctx:
        consts = ctx.enter_context(tc.tile_pool(name="consts", bufs=1))
        efp = ctx.enter_context(tc.tile_pool(name="efp", bufs=4))
        adjp = ctx.enter_context(tc.tile_pool(name="adjp", bufs=1))
        work = ctx.enter_context(tc.tile_pool(name="work", bufs=2))
        outp = ctx.enter_context(tc.tile_pool(name="outp", bufs=2))
        pset = ctx.enter_context(tc.tile_pool(name="pset", bufs=1, space="PSUM"))
        pmm = ctx.enter_context(tc.tile_pool(name="pmm", bufs=1, space="PSUM"))
        pms = ctx.enter_context(tc.tile_pool(name="pms", bufs=2, space="PSUM"))

        def emit_body():
            ident = consts.tile([128, 128], F32)
            make_identity(nc, ident)

            # --- weights: transpose on PE; biases folded as extra matmul row
            # rhs_n = [Wn_w^T ; Wn_b + We_b]  (65, 64)
            # rhs_s = [Ws_w^T ; Ws_b]         (65, 64)
            # weT   = We_w^T                  (16, 64)
            wn_sb = consts.tile([OD, ND], F32, tag="wload")
            nc.scalar.dma_start(out=wn_sb, in_=wn_d.ap())
            ws_sb = consts.tile([OD, ND], F32, tag="wload2")
            nc.scalar.dma_start(out=ws_sb, in_=ws_d.ap())
            we_sb = consts.tile([OD, ED], F32, tag="wload3")
            nc.scalar.dma_start(out=we_sb, in_=we_d.ap())

            rhs_n = consts.tile([ND + 1, OD], F32)
            rhs_s = consts.tile([ND + 1, OD], F32)
            weT = consts.tile([ED, OD], F32)

            pw = pset.tile([ND, OD], F32, tag="pw")
            nc.tensor.transpose(pw, wn_sb, ident[:ND, :OD])
            nc.scalar.copy(out=rhs_n[0:ND, :], in_=pw)
            pw2 = pset.tile([ND, OD], F32, tag="pw")
            nc.tensor.transpose(pw2, ws_sb, ident[:ND, :OD])
            nc.scalar.copy(out=rhs_s[0:ND, :], in_=pw2)
            pw3 = pset.tile([ED, OD], F32, tag="pw")
            nc.tensor.transpose(pw3, we_sb, ident[:ND, :OD])
            nc.scalar.copy(out=weT, in_=pw3)

            bias_n = consts.tile([1, OD], F32)
            nc.scalar.dma_start(out=bias_n, in_=_row_ap(wnb_d, OD))
            bias_e = consts.tile([1, OD], F32)
            nc.scalar.dma_start(out=bias_e, in_=_row_ap(web_d, OD))
            nc.vector.tensor_add(rhs_n[ND : ND + 1, :], bias_n, bias_e)
            nc.scalar.dma_start(out=rhs_s[ND : ND + 1, :], in_=_row_ap(wsb_d, OD))

            # --- h^T with an appended ones-row: (65, 512) ---
            hT = consts.tile([ND + 1, N], F32)
            nc.vector.memset(hT[ND : ND + 1, :], 1.0)
            for ib in range(NBLK):
                h_sb = work.tile([PB, ND], F32, tag="hload")
                nc.scalar.dma_start(out=h_sb, in_=h_ap[ib * PB : (ib + 1) * PB, :])
                ph = pset.tile([ND, PB], F32, tag="ph")
                nc.tensor.transpose(ph, h_sb, ident)
                nc.scalar.copy(out=hT[0:ND, ib * PB : (ib + 1) * PB], in_=ph)

            # --- adj / degree prep for all blocks (off the ef critical path;
            #     scalar-engine HWDGE ring, distinct from the ef ring).  The
            #     row-sum runs on ACT via activation accum_out; the pn*degr
            #     scaling later also runs on ACT -- both off the DVE stream.
            adj_fs, rs, degrs = [], [], []
            act_scr = work.tile([PB, N], F32, tag="actscr")
            for ib in range(NBLK):
                i0 = ib * PB
                adj_i = adjp.tile([PB, N], I32, tag=f"adji{ib}")
                nc.scalar.dma_start(out=adj_i, in_=adj_ap[i0 : i0 + PB, :])
                adj_f = adjp.tile([PB, N], F32, tag=f"adjf{ib}")
                nc.vector.tensor_copy(out=adj_f, in_=adj_i)

                deg = work.tile([PB, 1], F32, tag=f"deg{ib}")
                nc.scalar.activation(
                    out=act_scr,
                    in_=adj_f,
                    func=mybir.ActivationFunctionType.Copy,
                    accum_out=deg,
                )
                degc = work.tile([PB, 1], F32, tag=f"degc{ib}")
                nc.vector.tensor_scalar_max(degc, deg, 1.0)
                r = work.tile([PB, 1], F32, tag=f"r{ib}")
                nc.vector.reciprocal(r, degc)
                degr = work.tile([PB, 1], F32, tag=f"degr{ib}")
                nc.scalar.activation(
                    out=degr, in_=deg,
                    func=mybir.ActivationFunctionType.Copy, scale=r,
                )
                adj_fs.append(adj_f)
                rs.append(r)
                degrs.append(degr)

            # --- main loop over destination-node blocks ---
            HJ = N // 2
            scratch = work.tile([PB, N], F32, tag="scratch")
            for ib in range(NBLK):
                i0 = ib * PB
                adj_f, r, degr = adj_fs[ib], rs[ib], degrs[ib]

                # j-halves as separate DMAs: bounded queue delay for the
                # small transfers sharing the DMA engines.
                ef_t = efp.tile([PB, N, ED], F32, tag="ef")
                nc.sync.dma_start(
                    out=ef_t[:, 0:HJ, :], in_=ef_ap[i0 : i0 + PB, 0:HJ, :]
                )
                nc.sync.dma_start(
                    out=ef_t[:, HJ:N, :], in_=ef_ap[i0 : i0 + PB, HJ:N, :]
                )

                # masked sum over all 512 source nodes j: one fused
                # multiply+reduce DVE op per e-channel (FD=512).
                msum = work.tile([PB, ED], F32, tag="msum")
                for e in range(ED):
                    nc.vector.scalar_tensor_tensor(
                        out=scratch,
                        in0=ef_t[:, :, e],
                        scalar=1.0,
                        in1=adj_f,
                        op0=mybir.AluOpType.bypass,
                        op1=mybir.AluOpType.mult,
                        accum_out=msum[:, e : e + 1],
                    )
                ms = work.tile([PB, ED], F32, tag="ms")
                nc.vector.tensor_scalar_mul(ms, msum, r)

                # (128, 16) -> (16, 128) for the We projection
                pm = pms.tile([ED, PB], F32, tag="pm")
                nc.tensor.transpose(pm, ms, ident)
                msT = work.tile([ED, PB], F32, tag="msT")
                nc.scalar.copy(out=msT, in_=pm)

                # psum_es = (r*ms)^T We^T + h Ws^T + Ws_b   (PSUM accumulate)
                pes = pmm.tile([PB, OD], F32, tag="pes")
                nc.tensor.matmul(pes, lhsT=msT, rhs=weT, start=True, stop=False)
                nc.tensor.matmul(
                    pes, lhsT=hT[:, i0 : i0 + PB], rhs=rhs_s, start=False, stop=True
                )
                pn = pmm.tile([PB, OD], F32, tag="pn")
                nc.tensor.matmul(
                    pn, lhsT=hT[:, i0 : i0 + PB], rhs=rhs_n, start=True, stop=True
                )

                # out = relu(degr * node + pes); the scale runs on ACT
                acc = outp.tile([PB, OD], F32, tag="acc")
                nc.scalar.activation(
                    out=acc, in_=pn,
                    func=mybir.ActivationFunctionType.Copy, scale=degr,
                )
                ob = outp.tile([PB, OD], F32, tag="ob")
                nc.vector.scalar_tensor_tensor(
                    out=ob,
                    in0=pes,
                    scalar=1.0,
                    in1=acc,
                    op0=mybir.AluOpType.bypass,
                    op1=mybir.AluOpType.add,
                )
                nc.scalar.activation(
                    out=ob, in_=ob, func=mybir.ActivationFunctionType.Relu
                )
                nc.scalar.dma_start(out=out_ap[i0 : i0 + PB, :], in_=ob)

        if repeat == 1:
            for _ in range(unroll):
                emit_body()
        else:
            with tc.For_i(0, repeat, 1):
                for _ in range(unroll):
                    emit_body()

    nc.compile()
    return nc


_NC_CACHE = None


def _get_nc():
    global _NC_CACHE
    if _NC_CACHE is None:
        _NC_CACHE = build_bass()
    return _NC_CACHE


def make_in_maps(inputs):
    w = {
        k: np.ascontiguousarray(np.asarray(inputs[k], dtype=np.float32))
        for k in ("Wn_w", "Wn_b", "We_w", "We_b", "Ws_w", "Ws_b")
    }
    h = np.asarray(inputs["h"], dtype=np.float32)
    adj = np.asarray(inputs["adj"], dtype=np.int32)
    ef = np.asarray(inputs["edge_feat"], dtype=np.float32)
    in_maps = []
    for c in range(NCORES):
        m = dict(w)
        m["h"] = np.ascontiguousarray(h[c])
        m["adj"] = np.ascontiguousarray(adj[c])
        m["edge_feat"] = np.ascontiguousarray(ef[c])
        in_maps.append(m)
    return in_maps


def run(inputs, trace=False):
    """Run on hardware; returns (full_output, BassKernelResults)."""
    nc = _get_nc()
    res = run_bass_kernel_spmd(nc, make_in_maps(inputs), list(range(NCORES)), trace=trace)
    out = np.stack(
        [np.asarray(res.results[c]["out"]) for c in range(NCORES)], axis=0
    ).astype(np.float32)
    return out, res


def kernel(**inputs):
    out, _ = run(inputs)
    return out
